# revision 22
# baseline (speedup 1.0000x reference)
"""Trainium2 Bass kernel for nn_Discriminator_AddDim_ESSAAttn.

Network (per sample, C=128, 27x27 spatial, N=729 tokens):
  ESSA linear attention -> concat -> 1x1-conv FFN (+residual) ->
  3x3 conv/relu/pool x2 -> 3 FC layers -> [16] logits.
Batch 256 sharded 32-per-core across 8 NeuronCores (data parallel).

Key optimizations over the straightforward formulation:
  - x_essa is only consumed by concat->ffn1, so w_ln/w1a fold:
      h = leaky((w1x + wv@wln@w1a)^T x + (kvsb @ wln@w1a)^T q2nT + b1')
    eliminating the t2/attn matmuls and their extracts entirely.
    M4 = kvsb^T-transpose @ wla2 is a 64-col matmul per sample.
  - q2 row-normalisation simplifies exactly: q2n = q2 / sqrt(sum(q2^2))
    (the 1/(sum+eps) factor cancels in the L2 norm), killing the sq2
    reduce chain.
  - channel-mean subtraction of q/k folded into the qkv weights.
  - k2 column (token) L2 norm folded into the kv output scale (invs)
    computed from the gram diag in the same PE pass.
  - elementwise work spread across Act/DVE/Pool(gpsimd) engines; the
    Pool engine (idle otherwise) takes v-copies, q2n scaling, psum
    extracts, xen residual and all maxpools.
  - ffn1 psum pair-packed [128,768] (two samples in partition halves),
    extracted by single Act/DVE ops; ffn2 uses a partition-duplicated
    w2t so each sample's matmul reads its own half.
  - conv windows trimmed: conv1 26-wide (1 garbage col), conv2 10-wide
    (garbage-free); conv2 batches 4 samples per matmul.
  - attention tail (q2n/kvsb/M4/q2nT) runs in bf16 (PE transposes at
    1 cyc/row, halved extract traffic).
"""
import sys

sys.path.insert(0, "/opt/trn_rl_repo")

import numpy as np

import concourse.bass as bass
import concourse.tile as tile
from concourse import mybir
from concourse.bass_utils import run_bass_kernel_spmd

F32 = mybir.dt.float32
F32R = mybir.dt.float32r
BF16 = mybir.dt.bfloat16
AF = mybir.ActivationFunctionType
ALU = mybir.AluOpType
AX = mybir.AxisListType

N_CORES = 8
B, C, P = 256, 128, 27
NTOK = P * P          # 729
S = B // N_CORES      # 32 samples per core
NT = 6                # token tiles: 5*128 + 89
TOK_SIZES = [128, 128, 128, 128, 128, 89]
CGRP = 4              # conv2 sample-group size

# feature flags for walrus-compat bisection
USE_POOL_MUL = False   # q4/k2a/q2n broadcast muls on gpsimd
USE_POOL_STT = False   # leaky relu on gpsimd
USE_POOL_MAX = False   # maxpools on gpsimd
USE_TTR = False        # tensor_tensor_reduce for gram diag
USE_BCAST = True       # stride-0 broadcast APs


def _split_waits(nc, maxw=1):
    """walrus CoreV3 rejects instructions carrying >1 sem-wait; hoist
    extras onto preceding same-engine no-op carriers."""
    import bass_rust

    for bb in nc.m.functions[0].blocks:
        newlist = []
        for ins in bb.instructions:
            sw = ins.sync_info
            if sw and sw.on_wait and len(sw.on_wait) > maxw:
                waits = list(sw.on_wait)
                keep = waits[-maxw:]
                hoist = waits[:-maxw]
                for i in range(0, len(hoist), maxw):
                    chunk = hoist[i : i + maxw]
                    nop = bass_rust.InstNoOp(
                        name=f"{ins.name}_wsplit{i}", ins=[], outs=[]
                    )
                    nop.engine = ins.engine
                    nop.sync_info = mybir.SyncInfo(on_wait=list(chunk), on_update=[])
                    nc.register_instruction(nop, overwrite=True)
                    newlist.append(nop)
                ins.sync_info = mybir.SyncInfo(
                    on_wait=list(keep), on_update=list(sw.on_update)
                )
            newlist.append(ins)
        bb.instructions[:] = newlist


def _prep_weights(inputs):
    """Host-side weight massaging (all cheap numpy)."""
    f = lambda a: np.ascontiguousarray(np.asarray(a, np.float32))
    w_qkv = f(inputs["w_qkv"]).copy()          # [128, 384]
    b_qkv = f(inputs["b_qkv"]).copy()          # [384]
    # fold channel-mean subtraction of q and k into the weights/bias
    w_qkv[:, 0:128] -= w_qkv[:, 0:128].mean(axis=1, keepdims=True)
    w_qkv[:, 128:256] -= w_qkv[:, 128:256].mean(axis=1, keepdims=True)
    b_qkv[0:128] -= b_qkv[0:128].mean()
    b_qkv[128:256] -= b_qkv[128:256].mean()

    wln = f(inputs["w_ln"])                    # [128, 128]
    bln = f(inputs["b_ln"])                    # [128]
    w_ffn1 = f(inputs["w_ffn1"]).reshape(64, 256)     # [out, in]
    w1x = np.ascontiguousarray(w_ffn1[:, 0:128].T)    # [128, 64]
    w1a = np.ascontiguousarray(w_ffn1[:, 128:256].T)  # [128, 64]
    wv = w_qkv[:, 256:384]                            # [128, 128]
    bv = b_qkv[256:384]                               # [128]
    # fold w_ln (+biases) of the essa output into ffn1
    wla2 = np.ascontiguousarray(wln @ w1a)            # [128, 64]
    W_A = np.ascontiguousarray(w1x + wv @ wla2)       # [128, 64]
    b1p = f(inputs["b_ffn1"]).ravel() + w1a.T @ bln + wla2.T @ bv  # [64]
    b1dup = np.concatenate([b1p, b1p]).reshape(-1, 1)  # [128, 1]

    w2t = f(inputs["w_ffn2"]).reshape(128, 64).T       # [64, 128]
    w2tdup = np.ascontiguousarray(np.concatenate([w2t, w2t], axis=0))  # [128,128]

    # conv taps -> [in_ch, 9, out_ch]
    wc1 = np.ascontiguousarray(
        f(inputs["w_c1"]).transpose(2, 3, 1, 0).reshape(9, 128, 64).transpose(1, 0, 2)
    )  # [128, 9, 64]
    wc2 = np.ascontiguousarray(
        f(inputs["w_c2"]).transpose(2, 3, 1, 0).reshape(9, 64, 128).transpose(1, 0, 2)
    )  # [64, 9, 128]

    w1r = np.ascontiguousarray(f(inputs["w_fc1"]).reshape(128, 25, 512))
    wf2 = np.ascontiguousarray(f(inputs["w_fc2"]).reshape(4, 128, 512).transpose(1, 0, 2))
    wcls = np.ascontiguousarray(f(inputs["w_cls"]).reshape(4, 128, 16).transpose(1, 0, 2))

    col = lambda a: np.ascontiguousarray(f(a).reshape(-1, 1))
    row = lambda a: np.ascontiguousarray(f(a).reshape(1, -1))
    w = {
        "wqkv": w_qkv,
        "bqkv_row": row(b_qkv),
        "wla2": wla2,
        "W_A": W_A,
        "b1dup": np.ascontiguousarray(b1dup, dtype=np.float32),
        "w2tdup": w2tdup,
        "b2": col(inputs["b_ffn2"]),
        "wc1": wc1,
        "bc1": col(inputs["b_c1"]),
        "wc2": wc2,
        "bc2": col(inputs["b_c2"]),
        "w1r": w1r,
        "b1row": row(inputs["b_fc1"]),
        "wf2": wf2,
        "b2row": row(inputs["b_fc2"]),
        "wcls": wcls,
        "bcrow": row(inputs["b_cls"]),
        "eye": np.eye(128, dtype=np.float32),
        "eyeb": np.eye(128, dtype=np.float32),
        "ones1": np.ones((1, S), dtype=np.float32),
    }
    flags = {
        "qkv_bias": bool(np.any(b_qkv)),
        "fc1_bias": bool(np.any(w["b1row"])),
        "fc2_bias": bool(np.any(w["b2row"])),
        "cls_bias": bool(np.any(w["bcrow"])),
    }
    return w, flags


class _W:
    pass


_F32_WEIGHTS = {"b2", "bc1", "bc2", "b1dup"}      # activation/STT bias operands
_BF16_WEIGHTS = {"wla2", "w2tdup", "wc1", "wc2", "eyeb"}  # bf16 matmul path


def _load_weights(nc, pool, wvals):
    """Declare dram params + DMA every weight into resident SBUF tiles.
    bf16 weights are cast during a gpsimd DMA (only engine that casts)."""
    W = _W()
    for name, arr in wvals.items():
        if name in _F32_WEIGHTS:
            dt = F32
        elif name in _BF16_WEIGHTS:
            dt = BF16
        else:
            dt = F32R
        dram = nc.declare_dram_parameter(
            name, list(arr.shape), F32 if dt == BF16 else dt, isOutput=False
        )
        t = pool.tile(list(arr.shape), dt, name=f"sb_{name}")
        if dt == BF16:
            nc.gpsimd.dma_start(out=t, in_=dram[:])
        else:
            nc.sync.dma_start(out=t, in_=dram[:])
        setattr(W, name, t)
    return W


def _win(ap, offset, dims):
    """Manual sub-AP of a tile: dims = [[stride, count], ...] free dims."""
    return bass.AP(
        tensor=ap.tensor, offset=ap.offset + offset,
        ap=[list(ap.ap[0])] + [list(d) for d in dims],
    )


def _bc(ap2, n_inner):
    """[128, NT] stats AP -> [128, NT, n_inner] stride-0 broadcast."""
    return bass.AP(
        tensor=ap2.tensor, offset=ap2.offset,
        ap=[list(ap2.ap[0]), list(ap2.ap[1]), [0, n_inner]],
    )


def _mm(nc, out, lhsT, rhs, start=True, stop=True):
    nc.tensor.matmul(out, lhsT, rhs, start=start, stop=stop)


def _tp(nc, out, in_, eye):
    nc.tensor.matmul(
        out.bitcast(in_.dtype), in_, eye.bitcast(in_.dtype), is_transpose=True
    )


def _s0(nc, pools, W, flags, x_dram, s):
    """qkv + token stats; leaves q2n (bf16) and kvsrc (v|k2a) ready."""
    acts, stats, psum = pools["acts"], pools["stats"], pools["psum"]
    st = {"s": s}

    x_s = acts.tile([C, 768], F32R, name="x_s", bufs=5)
    nc.sync.dma_start(out=x_s[:, 0:NTOK], in_=x_dram[s])
    st["x_s"] = x_s

    qk2 = acts.tile([128, NT, 256], F32, name="qk2", bufs=4)
    kvsrc = acts.tile([128, NT, 256], F32R, name="kvsrc", bufs=5)
    for g in range(3):
        pq = psum.tile([128, 2, 512], F32, name="pq", tag="pqkv", bufs=1)
        for i in range(2):
            t = 2 * g + i
            nt = TOK_SIZES[t]
            _mm(nc, pq[0:nt, i, 0:384], x_s[:, 128 * t : 128 * t + nt], W.wqkv,
                start=True, stop=not flags["qkv_bias"])
            if flags["qkv_bias"]:
                _mm(nc, pq[0:nt, i, 0:384], W.ones1[0:1, 0:nt], W.bqkv_row,
                    start=False, stop=True)
        h2 = slice(2 * g, 2 * g + 2)
        # q^2 and k^2 in one pass (adjacent psum cols)
        nc.scalar.activation(qk2[:, h2, :], pq[:, :, 0:256], AF.Square)
        nc.scalar.copy(kvsrc[:, h2, 0:128], pq[:, :, 256:384])
    q2 = qk2[:, :, 0:128]
    k2 = qk2[:, :, 128:256]

    # k-path: k2a = k2 / (sum_c k2 + 1e-7), written into kvsrc cols 128:256
    sk2 = stats.tile([128, NT], F32, name="sk2", bufs=3)
    nc.vector.reduce_sum(sk2, k2, axis=AX.X)
    s1k = stats.tile([128, NT], F32, name="s1k", bufs=3)
    nc.vector.tensor_scalar_add(s1k, sk2, 1e-7)
    nc.vector.reciprocal(s1k, s1k)
    eng_mul = nc.gpsimd if USE_POOL_MUL else nc.vector
    if USE_BCAST:
        eng_mul.tensor_mul(kvsrc[:, :, 128:256], k2, _bc(s1k, 128))
    else:
        for t in range(NT):
            eng_mul.tensor_scalar_mul(kvsrc[:, t, 128:256], k2[:, t, :], s1k[:, t : t + 1])

    # q-path: q2n = q2 / sqrt(sum_c q2^2)   (exact simplification)
    q4 = acts.tile([128, NT, 128], F32, name="q4", bufs=2)
    nc.scalar.activation(q4, q2, AF.Square)
    sq4 = stats.tile([128, NT], F32, name="sq4", bufs=3)
    nc.vector.reduce_sum(sq4, q4, axis=AX.X)
    r4 = stats.tile([128, NT], F32, name="r4", bufs=3)
    nc.vector.reciprocal(r4, sq4)
    cq = stats.tile([128, NT], F32, name="cq", bufs=3)
    nc.scalar.activation(cq, r4, AF.Sqrt)
    q2n = acts.tile([128, NT, 128], BF16, name="q2n", bufs=5)
    if USE_BCAST:
        (nc.gpsimd if USE_POOL_MUL else nc.vector).tensor_mul(q2n, q2, _bc(cq, 128))
    else:
        for t in range(NT):
            (nc.gpsimd if USE_POOL_MUL else nc.vector).tensor_scalar_mul(
                q2n[:, t, :], q2[:, t, :], cq[:, t : t + 1])
    st.update(q2n=q2n, kvsrc=kvsrc)
    return st


def _s1_kv(nc, pools, W, st):
    """kv gram matmuls + column-norm scale -> kvsb (bf16)."""
    acts, stats, psum = pools["acts"], pools["stats"], pools["psum"]
    kvsrc = st["kvsrc"]
    pkv = psum.tile([128, 256], F32, name="pkv", tag="pA", bufs=2)
    for t in range(NT):
        nt = TOK_SIZES[t]
        _mm(nc, pkv, kvsrc[0:nt, t, 128:256], kvsrc[0:nt, t, :],
            start=(t == 0), stop=(t == NT - 1))
    tmpd = acts.tile([128, 128], F32, name="tmpd", bufs=2)
    s2 = stats.tile([128, 1], F32, name="s2", bufs=3)
    if USE_TTR:
        nc.vector.tensor_tensor_reduce(
            out=tmpd, in0=pkv[:, 128:256], in1=W.eye, scale=1.0, scalar=0.0,
            op0=ALU.mult, op1=ALU.add, accum_out=s2,
        )
    else:
        nc.vector.tensor_mul(tmpd, pkv[:, 128:256], W.eye)
        nc.vector.reduce_sum(s2, tmpd, axis=AX.X)
    invs = stats.tile([128, 1], F32, name="invs", bufs=3)
    nc.scalar.activation(invs, s2, AF.Sqrt, scale=float(NTOK))  # 27*sqrt(s2)
    nc.vector.reciprocal(invs, invs)
    kvsb = acts.tile([128, 128], BF16, name="kvsb", bufs=3)
    nc.vector.tensor_scalar_mul(kvsb, pkv[:, 0:128], invs)
    st["kvsb"] = kvsb


def _s1_tp(nc, pools, W, st):
    """PE-transpose q2n (bf16) -> q2nT."""
    acts, psum = pools["acts"], pools["psum"]
    q2n = st["q2n"]
    pqt = psum.tile([128, 384], F32, name="pqt", tag="pQ", bufs=2)
    for t in range(NT):
        _tp(nc, pqt[:, 64 * t : 64 * (t + 1)], q2n[:, t, :], W.eyeb)
    q2nT = acts.tile([128, 768], BF16, name="q2nT", bufs=4)
    nc.scalar.copy(q2nT, pqt[:, 0:384].bitcast(BF16))
    st["q2nT"] = q2nT


def _s1_m4a(nc, pools, W, st):
    """Transpose kvsb."""
    acts, psum = pools["acts"], pools["psum"]
    pkvT = psum.tile([128, 64], F32, name="pkvT", tag="pA", bufs=2)
    _tp(nc, pkvT[:, 0:64], st["kvsb"], W.eyeb)
    kvsbT = acts.tile([128, 128], BF16, name="kvsbT", bufs=3)
    nc.scalar.copy(kvsbT, pkvT[:, 0:64].bitcast(BF16))
    st["kvsbT"] = kvsbT


def _s1_m4b(nc, pools, W, st):
    """M4 = kvsb @ wla2 via the transposed kvsb."""
    acts, psum = pools["acts"], pools["psum"]
    pM4 = psum.tile([128, 64], F32, name="pM4", tag="pA", bufs=2)
    _mm(nc, pM4, st["kvsbT"], W.wla2)
    M4sb = acts.tile([128, 64], BF16, name="M4sb", bufs=3)
    nc.scalar.copy(M4sb, pM4)
    st["M4sb"] = M4sb


def _s2_ffn1(nc, pools, W, st):
    """ffn1: h = leaky(W_A^T x + M4^T q2nT + b1')."""
    acts, psum = pools["acts"], pools["psum"]
    ph = psum.tile([64, 768], F32, name="ph", tag="ph", bufs=1)
    _mm(nc, ph[:, 0:512], W.W_A, st["x_s"][:, 0:512], start=True, stop=False)
    _mm(nc, ph[:, 512:768], W.W_A, st["x_s"][:, 512:768], start=True, stop=False)
    _mm(nc, ph[:, 0:512], st["M4sb"], st["q2nT"][:, 0:512], start=False, stop=True)
    _mm(nc, ph[:, 512:768], st["M4sb"], st["q2nT"][:, 512:768], start=False, stop=True)
    h0p = acts.tile([64, 732], F32, name="h0p", bufs=2)
    nc.scalar.activation(h0p[:, 0:NTOK], ph[:, 0:NTOK], AF.Identity,
                         bias=W.b1dup[0:64])
    hp = acts.tile([64, 732], BF16, name="hp", bufs=2)
    (nc.gpsimd if USE_POOL_STT else nc.vector).scalar_tensor_tensor(
        out=hp[:, 0:NTOK], in0=h0p[:, 0:NTOK], scalar=0.01,
        in1=h0p[:, 0:NTOK], op0=ALU.mult, op1=ALU.max,
    )
    st["hp"] = hp


def _s2_ffn2(nc, pools, W, st):
    """ffn2 + residual."""
    acts, psum = pools["acts"], pools["psum"]
    hp = st["hp"]
    x_s = st["x_s"]
    pxen1 = psum.tile([128, 512], F32, name="pxen1", tag="pA", bufs=2)
    pxen2 = psum.tile([128, 220], F32, name="pxen2", tag="pA", bufs=2)
    _mm(nc, pxen1, W.w2tdup[0:64, :], hp[:, 0:512])
    _mm(nc, pxen2, W.w2tdup[0:64, :], hp[:, 512:732])
    xen = acts.tile([C, 768], BF16, name="xen", bufs=4)
    nc.vector.scalar_tensor_tensor(
        out=xen[:, 0:512], in0=pxen1, scalar=W.b2, in1=x_s[:, 0:512],
        op0=ALU.add, op1=ALU.add,
    )
    nc.vector.scalar_tensor_tensor(
        out=xen[:, 512:NTOK], in0=pxen2[:, 0:217], scalar=W.b2,
        in1=x_s[:, 512:NTOK], op0=ALU.add, op1=ALU.add,
    )
    st["xen"] = xen


def _s2_c1(nc, pools, W, st, grp, taps):
    """conv1 (26-wide windows) + relu + maxpool for one sample."""
    acts, psum = pools["acts"], pools["psum"]
    xen, s = st["xen"], st["s"]
    pc1a = psum.tile([64, 13, 28], F32, name="pc1a", tag="pA", bufs=2)
    pc1b = psum.tile([64, 12, 28], F32, name="pc1b", tag="pA", bufs=2)
    for ky in range(3):
        for kx in range(3):
            tap = ky * 3 + kx
            _mm(nc, pc1a, W.wc1[:, tap, :],
                _win(xen, ky * 27 + kx, [[27, 13], [1, 28]]),
                start=(tap == 0), stop=(tap == 8))
            _mm(nc, pc1b, W.wc1[:, tap, :],
                _win(xen, (ky + 13) * 27 + kx, [[27, 12], [1, 28]]),
                start=(tap == 0), stop=(tap == 8))
    o1r = acts.tile([64, 625], BF16, name="o1r", bufs=2)
    o1rv = o1r.rearrange("p (h w) -> p h w", h=25)
    nc.scalar.activation(o1rv[:, 0:13, :], pc1a[:, :, 0:25], AF.Relu, bias=W.bc1)
    nc.scalar.activation(o1rv[:, 13:25, :], pc1b[:, :, 0:25], AF.Relu, bias=W.bc1)
    o1r3 = o1r.rearrange("p (h w) -> p h w", h=25)
    m1 = acts.tile([64, 144], BF16, name="m1", bufs=2)
    m1v = m1.rearrange("p (a b) -> p a b", a=12)
    m2 = acts.tile([64, 144], BF16, name="m2", bufs=2)
    m2v = m2.rearrange("p (a b) -> p a b", a=12)
    g = s % CGRP
    o1pv = grp["o1p"][:, g, 0:144].rearrange("p (a b) -> p a b", a=12)
    eng_max = nc.gpsimd if USE_POOL_MAX else nc.vector
    eng_max.tensor_max(m1v, o1r3[:, 0:24:2, 0:24:2], o1r3[:, 0:24:2, 1:25:2])
    eng_max.tensor_max(m2v, o1r3[:, 1:25:2, 0:24:2], o1r3[:, 1:25:2, 1:25:2])
    eng_max.tensor_max(o1pv, m1v, m2v)

    if taps is not None and s == 0:
        for nm, t in (
            ("q2n", st["q2n"]), ("kvsb", st["kvsb"]), ("kvsbT", st["kvsbT"]),
            ("M4sb", st["M4sb"]), ("q2nT", st["q2nT"]), ("hp", st.get("hp")),
            ("xen", xen), ("o1p", grp["o1p"][:, 0, :]),
        ):
            if t is None:
                continue
            d = nc.declare_dram_parameter(f"tap_{nm}", list(t.shape), t.dtype, isOutput=True)
            nc.sync.dma_start(out=d[:], in_=t)
            taps.append(f"tap_{nm}")


def _emit_conv2_group(nc, pools, W, O2buf, grp, g0, gn):
    """conv2+pool for a group of gn samples (10-wide garbage-free)."""
    acts, psum = pools["acts"], pools["psum"]
    pc2 = psum.tile([128, CGRP, 10, 12], F32, name="pc2", tag="pA", bufs=2)
    for ky in range(3):
        for kx in range(3):
            tap = ky * 3 + kx
            _mm(nc, pc2[:, 0:gn], W.wc2[:, tap, :],
                _win(grp["o1p"], ky * 12 + kx, [[148, gn], [12, 10], [1, 12]]),
                start=(tap == 0), stop=(tap == 8))
    o2r = acts.tile([128, CGRP, 100], BF16, name="o2r", bufs=2)
    o2rv = o2r.rearrange("p g (h w) -> p g h w", h=10)
    nc.scalar.activation(o2rv[:, 0:gn], pc2[:, 0:gn, :, 0:10], AF.Relu, bias=W.bc2)
    n1 = acts.tile([128, CGRP, 25], F32, name="n1", bufs=2)
    n1v = n1.rearrange("p g (a b) -> p g a b", a=5)
    n2 = acts.tile([128, CGRP, 25], F32, name="n2", bufs=2)
    n2v = n2.rearrange("p g (a b) -> p g a b", a=5)
    eng_max = nc.gpsimd if USE_POOL_MAX else nc.vector
    eng_max.tensor_max(
        n1v[:, 0:gn], o2rv[:, 0:gn, 0:10:2, 0:10:2], o2rv[:, 0:gn, 0:10:2, 1:10:2]
    )
    eng_max.tensor_max(
        n2v[:, 0:gn], o2rv[:, 0:gn, 1:10:2, 0:10:2], o2rv[:, 0:gn, 1:10:2, 1:10:2]
    )
    outv = (
        O2buf[:, :, g0 : g0 + gn]
        .rearrange("p a g -> p g a")
        .rearrange("p g (a b) -> p g a b", a=5)
    )
    eng_max.tensor_max(outv, n1v[:, 0:gn], n2v[:, 0:gn])


def _emit_fc(nc, pools, W, flags, out_dram, O2buf, ns):
    psum, fc = pools["psum"], pools["fc"]
    ones = W.ones1[0:1, 0:ns]

    po3 = psum.tile([ns, 512], F32, name="po3", tag="pA", bufs=2)
    for p in range(25):
        _mm(nc, po3, O2buf[:, p, :], W.w1r[:, p, :],
            start=(p == 0), stop=(p == 24 and not flags["fc1_bias"]))
    if flags["fc1_bias"]:
        _mm(nc, po3, ones, W.b1row, start=False, stop=True)
    o3r = fc.tile([ns, 512], F32R, name="o3r")
    nc.scalar.activation(o3r, po3, AF.Relu)

    po3t = psum.tile([128, 4, ns], F32, name="po3t", tag="pA", bufs=2)
    for j in range(4):
        _tp(nc, po3t[:, j, :], o3r[:, 128 * j : 128 * (j + 1)], W.eye[0:ns, 0:ns])
    o3T = fc.tile([128, 4, ns], F32R, name="o3T")
    nc.vector.tensor_copy(o3T, po3t)

    po4 = psum.tile([ns, 512], F32, name="po4", tag="pA", bufs=2)
    for j in range(4):
        _mm(nc, po4, o3T[:, j, :], W.wf2[:, j, :],
            start=(j == 0), stop=(j == 3 and not flags["fc2_bias"]))
    if flags["fc2_bias"]:
        _mm(nc, po4, ones, W.b2row, start=False, stop=True)
    o4r = fc.tile([ns, 512], F32R, name="o4r")
    nc.scalar.activation(o4r, po4, AF.Relu)

    po4t = psum.tile([128, 4, ns], F32, name="po4t", tag="pA", bufs=2)
    for j in range(4):
        _tp(nc, po4t[:, j, :], o4r[:, 128 * j : 128 * (j + 1)], W.eye[0:ns, 0:ns])
    o4T = fc.tile([128, 4, ns], F32R, name="o4T")
    nc.vector.tensor_copy(o4T, po4t)

    pcls = psum.tile([ns, 512], F32, name="pcls", tag="pA", bufs=2)
    for j in range(4):
        _mm(nc, pcls[:, 0:16], o4T[:, j, :], W.wcls[:, j, :],
            start=(j == 0), stop=(j == 3 and not flags["cls_bias"]))
    if flags["cls_bias"]:
        _mm(nc, pcls[:, 0:16], ones, W.bcrow, start=False, stop=True)
    outsb = fc.tile([ns, 16], F32, name="outsb")
    nc.vector.tensor_copy(outsb, pcls[:, 0:16])
    nc.sync.dma_start(out=out_dram[:], in_=outsb)


def build_nc(wvals, flags, n_samples=S, debug=False):
    nc = bass.Bass()
    x_dram = nc.declare_dram_parameter("x", [n_samples, C, NTOK], F32R, isOutput=False)
    out_dram = nc.declare_dram_parameter("out", [n_samples, 16], F32, isOutput=True)
    taps = [] if debug else None

    with tile.TileContext(nc) as tc:
        with (
            tc.tile_pool(name="wts", bufs=1) as wts,
            tc.tile_pool(name="acts", bufs=2) as acts,
            tc.tile_pool(name="stats", bufs=3) as stats,
            tc.tile_pool(name="fc", bufs=1) as fc,
            tc.tile_pool(name="psum", bufs=1, space="PSUM") as psum,
        ):
            pools = {"acts": acts, "stats": stats, "psum": psum, "fc": fc}
            W = _load_weights(nc, wts, wvals)
            O2buf = fc.tile([128, 25, n_samples], F32R, name="O2buf")
            grp = {}  # group-index -> {"o1p": tile}

            def tail(sts):
                for st in sts:
                    _s1_kv(nc, pools, W, st)
                for st in sts:
                    _s1_tp(nc, pools, W, st)
                    _s1_m4a(nc, pools, W, st)
                for st in sts:
                    _s1_m4b(nc, pools, W, st)
                for st in sts:
                    _s2_ffn1(nc, pools, W, st)
                for st in sts:
                    _s2_ffn2(nc, pools, W, st)
                for st in sts:
                    _s2_c1(nc, pools, W, st, grp[st["s"] // CGRP], taps)
                s_last = sts[-1]["s"]
                if s_last % CGRP == CGRP - 1 or s_last == n_samples - 1:
                    g0 = (s_last // CGRP) * CGRP
                    _emit_conv2_group(
                        nc, pools, W, O2buf, grp[g0 // CGRP], g0, s_last - g0 + 1
                    )

            prev = None
            for p0 in range(0, n_samples, 2):
                pair = [p0] + ([p0 + 1] if p0 + 1 < n_samples else [])
                sts = []
                for s in pair:
                    if s % CGRP == 0:
                        grp[s // CGRP] = {
                            "o1p": acts.tile([64, CGRP, 148], BF16, name="o1p_grp", bufs=3)
                        }
                    sts.append(_s0(nc, pools, W, flags, x_dram, s))
                if prev is not None:
                    tail(prev)
                prev = sts
            tail(prev)
            _emit_fc(nc, pools, W, flags, out_dram, O2buf, n_samples)

    _split_waits(nc)
    return nc, taps


_BUILD_CACHE = {}


def kernel(**inputs):
    wvals, flags = _prep_weights(inputs)
    key = tuple(sorted(flags.items()))
    if key not in _BUILD_CACHE:
        _BUILD_CACHE[key] = build_nc(wvals, flags)
    nc, _ = _BUILD_CACHE[key]

    x = np.ascontiguousarray(np.asarray(inputs["x"], np.float32)).reshape(
        N_CORES, S, C, NTOK
    )
    in_maps = []
    for c in range(N_CORES):
        m = {"x": np.ascontiguousarray(x[c])}
        m.update(wvals)
        in_maps.append(m)
    last_err = None
    for _attempt in range(3):
        try:
            res = run_bass_kernel_spmd(nc, in_maps, core_ids=list(range(N_CORES)))
            break
        except Exception as e:  # transient device faults: retry
            last_err = e
    else:
        raise last_err
    out = np.concatenate([res.results[c]["out"] for c in range(N_CORES)], axis=0)
    return out.astype(np.float32)
